# revision 5
# baseline (speedup 1.0000x reference)
"""CrossCoder kernel for 8 Trainium2 NeuronCores (Bass/Tile, SPMD).

Math (reference):
    f     = relu(einsum('bld,ldf->bf', x, W_enc) + b_enc)     # [B, F]
    x_hat = einsum('bf,lfd->bld', f, W_dec) + b_dec           # [B, L, D]

Sharding: dict dim F=32768 split 8 ways (FL=4096 per core, tensor parallel
over latents). Each core computes its local f shard (encode), the partial
decode sum over its latents, then a ReduceScatter combines partials so core
i ends with rows [i*256:(i+1)*256) of the (LD=2048, B) transposed output.
The host concatenates the 8 shards and transposes back.

Layout on device is feature-major: encode output fT[f, b], decode output
x_hatT[l*D+d, b]; the contraction dim always sits on SBUF partitions.
b_dec/8 is folded into each core's partial pre-ReduceScatter.
All matmuls run as float32r (full PE rate, ~1e-4 rel err).
"""

import numpy as np

B = 1024
L = 2
D = 1024
F = 32768
NCORES = 8
FL = F // NCORES      # 4096 latents per core
LD = L * D            # 2048 (encode contraction / decode output rows)
KT = LD // 128        # 16 encode k-tiles
FT = FL // 128        # 32 f-tiles per core
NB = 512              # matmul moving free dim
RS_ROWS = LD // NCORES  # 256 output rows per core after ReduceScatter

_CACHE = {}


def _build_nc():
    import concourse.bass as bass  # noqa: F401
    import concourse.tile as tile
    from concourse import bacc, mybir

    f32 = mybir.dt.float32
    f32r = mybir.dt.float32r

    nc = bacc.Bacc()

    xT = nc.declare_dram_parameter("xT", [LD, B], f32r, isOutput=False)
    w_enc = nc.declare_dram_parameter("w_enc", [LD, FL], f32r, isOutput=False)
    w_dec = nc.declare_dram_parameter("w_dec", [L, FL, D], f32r, isOutput=False)
    b_enc = nc.declare_dram_parameter("b_enc", [FL], f32, isOutput=False)
    b_dec8 = nc.declare_dram_parameter("b_dec8", [LD], f32, isOutput=False)
    out_sh = nc.declare_dram_parameter("out_sh", [RS_ROWS, B], f32, isOutput=True)

    partial = nc.dram_tensor("partial", [LD, B], f32)
    rs_out = nc.dram_tensor("rs_out", [RS_ROWS, B], f32)

    xT_a = xT.ap()
    w_enc_a = w_enc.ap()
    w_dec_a = w_dec.ap()
    partial_a = partial.ap()

    with tile.TileContext(nc) as tc:
        with (
            tc.tile_pool(name="xp", bufs=2) as xp,
            tc.tile_pool(name="fp", bufs=1) as fp,
            tc.tile_pool(name="we", bufs=4) as we,
            tc.tile_pool(name="wd", bufs=4) as wd,
            tc.tile_pool(name="stg", bufs=4) as stg,
            tc.tile_pool(name="bias", bufs=1) as bias,
            tc.tile_pool(name="ps", bufs=8, space="PSUM") as ps,
        ):
            benc_t = bias.tile([128, FT], f32)
            nc.sync.dma_start(out=benc_t, in_=b_enc.ap().rearrange("(t p) -> p t", p=128))
            bdec_t = bias.tile([128, KT], f32)
            nc.sync.dma_start(out=bdec_t, in_=b_dec8.ap().rearrange("(t p) -> p t", p=128))

            for h in range(2):
                bsl = bass.ds(h * NB, NB)
                # resident rhs: xT half, 16 tiles [128k, 512b]
                x_tiles = []
                for k in range(KT):
                    xt = xp.tile([128, NB], f32r, tag=f"x{k}")
                    nc.sync.dma_start(out=xt, in_=xT_a[k * 128 : (k + 1) * 128, bsl])
                    x_tiles.append(xt)

                # ---- encode: fT[f, b] = relu(sum_k W_enc[k,f] * xT[k,b] + b_enc[f])
                f_tiles = []
                for fg in range(FT // 4):  # groups of 4 f-tiles share one weight DMA
                    pss = [ps.tile([128, NB], f32, tag="ps", name=f"pss{_j}") for _j in range(4)]
                    for k in range(KT):
                        wt = we.tile([128, 512], f32r)
                        nc.sync.dma_start(
                            out=wt,
                            in_=w_enc_a[k * 128 : (k + 1) * 128, fg * 512 : (fg + 1) * 512],
                        )
                        for j in range(4):
                            nc.tensor.matmul(
                                pss[j],
                                wt[:, j * 128 : (j + 1) * 128],
                                x_tiles[k],
                                start=(k == 0),
                                stop=(k == KT - 1),
                            )
                    for j in range(4):
                        ft_idx = fg * 4 + j
                        ftile = fp.tile([128, NB], f32r, tag=f"f{ft_idx}")
                        nc.scalar.activation(
                            ftile,
                            pss[j],
                            mybir.ActivationFunctionType.Relu,
                            bias=benc_t[:, ft_idx : ft_idx + 1],
                        )
                        f_tiles.append(ftile)

                # ---- decode: partial[l*D+d, b] = sum_f W_dec[l,f,d] * fT[f,b] + b_dec[l,d]/8
                for l in range(L):
                    for dg in range(2):  # d in blocks of 512 (4 ld-tiles each)
                        pss = [ps.tile([128, NB], f32, tag="ps", name=f"pss{_j}") for _j in range(4)]
                        for fk in range(FT):
                            wt = wd.tile([128, 512], f32r)
                            nc.sync.dma_start(
                                out=wt,
                                in_=w_dec_a[l, fk * 128 : (fk + 1) * 128, dg * 512 : (dg + 1) * 512],
                            )
                            for j in range(4):
                                nc.tensor.matmul(
                                    pss[j],
                                    wt[:, j * 128 : (j + 1) * 128],
                                    f_tiles[fk],
                                    start=(fk == 0),
                                    stop=(fk == FT - 1),
                                )
                        for j in range(4):
                            ld_t = l * 8 + dg * 4 + j  # ld tile index (0..15)
                            st = stg.tile([128, NB], f32)
                            nc.vector.tensor_scalar_add(
                                st, pss[j], bdec_t[:, ld_t : ld_t + 1]
                            )
                            nc.sync.dma_start(
                                out=partial_a[ld_t * 128 : (ld_t + 1) * 128, bsl],
                                in_=st,
                            )

    # ---- post-Tile: ReduceScatter partials across the 8 cores, then write out
    with (
        nc.Block() as block,
        nc.semaphore("cc_sem") as cc_sem,
        nc.semaphore("odma_sem") as odma_sem,
    ):

        @block.gpsimd
        def _(g):
            g.collective_compute(
                "ReduceScatter",
                mybir.AluOpType.add,
                ins=[partial[:]],
                outs=[rs_out[:]],
                replica_groups=[list(range(NCORES))],
            ).then_inc(cc_sem, 1)
            g.wait_ge(cc_sem, 1)
            g.dma_start(out=out_sh.ap(), in_=rs_out[:]).then_inc(odma_sem, 16)
            g.wait_ge(odma_sem, 16)

    nc.finalize()
    return nc


def _get_nc():
    if "nc" not in _CACHE:
        _CACHE["nc"] = _build_nc()
    return _CACHE["nc"]


def kernel(x, W_enc, b_enc, W_dec, b_dec):
    from concourse.bass_utils import run_bass_kernel_spmd

    x = np.asarray(x, dtype=np.float32)
    W_enc = np.asarray(W_enc, dtype=np.float32)
    b_enc = np.asarray(b_enc, dtype=np.float32)
    W_dec = np.asarray(W_dec, dtype=np.float32)
    b_dec = np.asarray(b_dec, dtype=np.float32)

    nc = _get_nc()

    xT = np.ascontiguousarray(x.reshape(B, LD).T)            # [LD, B]
    w_enc_flat = W_enc.reshape(LD, F)                        # [LD, F]
    bdec8 = (b_dec.reshape(LD) / NCORES).astype(np.float32)  # folded pre-RS

    in_maps = []
    for i in range(NCORES):
        fsl = slice(i * FL, (i + 1) * FL)
        in_maps.append(
            {
                "xT": xT,
                "w_enc": np.ascontiguousarray(w_enc_flat[:, fsl]),
                "w_dec": np.ascontiguousarray(W_dec[:, fsl, :]),
                "b_enc": np.ascontiguousarray(b_enc[fsl]),
                "b_dec8": bdec8,
            }
        )

    res = run_bass_kernel_spmd(nc, in_maps, list(range(NCORES)))
    _CACHE["last_res"] = res
    shards = [res.results[i]["out_sh"] for i in range(NCORES)]
    xhatT = np.concatenate(shards, axis=0)                   # [LD, B]
    return np.ascontiguousarray(xhatT.T).reshape(B, L, D).astype(np.float32)


# revision 6
# speedup vs baseline: 1.2883x; 1.2883x over previous
"""CrossCoder kernel for 8 Trainium2 NeuronCores (Bass/Tile, SPMD).

Math (reference):
    f     = relu(einsum('bld,ldf->bf', x, W_enc) + b_enc)     # [B, F]
    x_hat = einsum('bf,lfd->bld', f, W_dec) + b_dec           # [B, L, D]

Sharding: dict dim F=32768 split 8 ways (FL=4096 per core, tensor parallel
over latents). Each core computes its local f shard (encode), the partial
decode sum over its latents; ReduceScatters combine partials so core i ends
with rows [i*256:(i+1)*256) of the (LD=2048, B) transposed output. The host
concatenates the 8 shards and transposes back.

Device layout is feature-major (contraction dim on SBUF partitions); batch
is processed in two halves of 512; each half's partial is ReduceScattered
as soon as it is done, so the first collective overlaps the second half's
compute. Weights/x are host-repacked into contiguous [128, 512] tiles so
every DMA is a single 256KB contiguous block. b_dec/8 is folded in
pre-collective. All matmuls run as float32r (full PE rate, ~2e-4 rel err).
"""

import numpy as np

B = 1024
L = 2
D = 1024
F = 32768
NCORES = 8
FL = F // NCORES      # 4096 latents per core
LD = L * D            # 2048
KT = LD // 128        # 16 encode k-tiles
FT = FL // 128        # 32 f-tiles per core
NB = 512              # matmul moving free dim
NH = 2                # batch halves

_CACHE = {}


def _build_nc():
    import concourse.bass as bass  # noqa: F401
    import concourse.tile as tile
    from concourse import bacc, mybir

    f32 = mybir.dt.float32
    f32r = mybir.dt.float32r

    nc = bacc.Bacc()

    xT = nc.declare_dram_parameter("xT", [NH, KT, 128, NB], f32r, isOutput=False)
    w_enc = nc.declare_dram_parameter("w_enc", [KT, FT // 4, 128, NB], f32r, isOutput=False)
    w_dec = nc.declare_dram_parameter("w_dec", [L, 2, FT, 128, NB], f32r, isOutput=False)
    b_enc = nc.declare_dram_parameter("b_enc", [FL], f32, isOutput=False)
    b_dec8 = nc.declare_dram_parameter("b_dec8", [LD], f32, isOutput=False)
    out_sh = nc.declare_dram_parameter("out_sh", [NH, 2, 128, NB], f32, isOutput=True)

    partials = [nc.dram_tensor(f"partial{h}", [KT, 128, NB], f32) for h in range(NH)]
    rs_outs = [nc.dram_tensor(f"rs{h}", [2, 128, NB], f32) for h in range(NH)]

    xT_a = xT.ap()
    w_enc_a = w_enc.ap()
    w_dec_a = w_dec.ap()

    import contextlib

    with contextlib.ExitStack() as stack:
        cc_sem = stack.enter_context(nc.semaphore("cc_sem"))
        odma_sem = stack.enter_context(nc.semaphore("odma_sem"))

        for h in range(NH):
            partial_a = partials[h].ap()
            with tile.TileContext(nc) as tc:
                with (
                    tc.tile_pool(name=f"xp{h}", bufs=1) as xp,
                    tc.tile_pool(name=f"fp{h}", bufs=1) as fp,
                    tc.tile_pool(name=f"we{h}", bufs=14) as we,
                    tc.tile_pool(name=f"wd{h}", bufs=14) as wd,
                    tc.tile_pool(name=f"stg{h}", bufs=8) as stg,
                    tc.tile_pool(name=f"bias{h}", bufs=1) as bias,
                    tc.tile_pool(name=f"ps{h}", bufs=8, space="PSUM") as ps,
                ):
                    benc_t = bias.tile([128, FT], f32, name="benc")
                    nc.sync.dma_start(
                        out=benc_t, in_=b_enc.ap().rearrange("(t p) -> p t", p=128)
                    )
                    bdec_t = bias.tile([128, KT], f32, name="bdec")
                    nc.sync.dma_start(
                        out=bdec_t, in_=b_dec8.ap().rearrange("(t p) -> p t", p=128)
                    )

                    x_tiles = []
                    for k in range(KT):
                        xt = xp.tile([128, NB], f32r, tag=f"x{k}", name=f"x{k}")
                        nc.sync.dma_start(out=xt, in_=xT_a[h, k])
                        x_tiles.append(xt)

                    # ---- encode
                    f_tiles = []
                    for fg in range(FT // 4):
                        pss = [
                            ps.tile([128, NB], f32, tag="ps", name=f"pse{_j}")
                            for _j in range(4)
                        ]
                        for k in range(KT):
                            wt = we.tile([128, NB], f32r, tag="we", name="wet")
                            nc.sync.dma_start(out=wt, in_=w_enc_a[k, fg])
                            for j in range(4):
                                nc.tensor.matmul(
                                    pss[j],
                                    wt[:, j * 128 : (j + 1) * 128],
                                    x_tiles[k],
                                    start=(k == 0),
                                    stop=(k == KT - 1),
                                )
                        for j in range(4):
                            ft_idx = fg * 4 + j
                            ftile = fp.tile(
                                [128, NB], f32r, tag=f"f{ft_idx}", name=f"f{ft_idx}"
                            )
                            nc.scalar.activation(
                                ftile,
                                pss[j],
                                mybir.ActivationFunctionType.Relu,
                                bias=benc_t[:, ft_idx : ft_idx + 1],
                            )
                            f_tiles.append(ftile)

                    # ---- decode
                    for l in range(L):
                        for dg in range(2):
                            pss = [
                                ps.tile([128, NB], f32, tag="ps", name=f"psd{_j}")
                                for _j in range(4)
                            ]
                            for fk in range(FT):
                                wt = wd.tile([128, NB], f32r, tag="wd", name="wdt")
                                nc.sync.dma_start(out=wt, in_=w_dec_a[l, dg, fk])
                                for j in range(4):
                                    nc.tensor.matmul(
                                        pss[j],
                                        wt[:, j * 128 : (j + 1) * 128],
                                        f_tiles[fk],
                                        start=(fk == 0),
                                        stop=(fk == FT - 1),
                                    )
                            for j in range(4):
                                ld_t = l * 8 + dg * 4 + j
                                st = stg.tile([128, NB], f32, tag="st", name="st")
                                nc.vector.tensor_scalar_add(
                                    st, pss[j], bdec_t[:, ld_t : ld_t + 1]
                                )
                                nc.sync.dma_start(out=partial_a[ld_t], in_=st)

            # kick this half's ReduceScatter (no wait → overlaps next half)
            with nc.Block() as block:

                @block.gpsimd
                def _(g, _h=h):
                    g.collective_compute(
                        "ReduceScatter",
                        mybir.AluOpType.add,
                        ins=[partials[_h][:]],
                        outs=[rs_outs[_h][:]],
                        replica_groups=[list(range(NCORES))],
                    ).then_inc(cc_sem, 1)

        with nc.Block() as block:

            @block.gpsimd
            def _(g):
                g.wait_ge(cc_sem, NH)
                for h in range(NH):
                    g.dma_start(out=out_sh.ap()[h], in_=rs_outs[h][:]).then_inc(
                        odma_sem, 16
                    )
                g.wait_ge(odma_sem, 16 * NH)

    nc.finalize()
    return nc


def _get_nc():
    if "nc" not in _CACHE:
        _CACHE["nc"] = _build_nc()
    return _CACHE["nc"]


def kernel(x, W_enc, b_enc, W_dec, b_dec):
    from concourse.bass_utils import run_bass_kernel_spmd

    x = np.asarray(x, dtype=np.float32)
    W_enc = np.asarray(W_enc, dtype=np.float32)
    b_enc = np.asarray(b_enc, dtype=np.float32)
    W_dec = np.asarray(W_dec, dtype=np.float32)
    b_dec = np.asarray(b_dec, dtype=np.float32)

    nc = _get_nc()

    # xT blocked: [h, k, p, c] with xT row k*128+p (= x.reshape(B,LD).T), col h*512+c
    xT = np.ascontiguousarray(
        x.reshape(B, LD).T.reshape(KT, 128, NH, NB).transpose(2, 0, 1, 3)
    )
    w_enc_flat = W_enc.reshape(LD, F)
    bdec8 = (b_dec.reshape(LD) / NCORES).astype(np.float32)

    in_maps = []
    for i in range(NCORES):
        fsl = slice(i * FL, (i + 1) * FL)
        we_blk = np.ascontiguousarray(
            w_enc_flat[:, fsl].reshape(KT, 128, FT // 4, NB).transpose(0, 2, 1, 3)
        )
        wd_blk = np.ascontiguousarray(
            W_dec[:, fsl, :].reshape(L, FT, 128, 2, NB).transpose(0, 3, 1, 2, 4)
        )
        in_maps.append(
            {
                "xT": xT,
                "w_enc": we_blk,
                "w_dec": wd_blk,
                "b_enc": np.ascontiguousarray(b_enc[fsl]),
                "b_dec8": bdec8,
            }
        )

    res = run_bass_kernel_spmd(nc, in_maps, list(range(NCORES)))
    _CACHE["last_res"] = res
    # out_sh [h, t, p, c] → shard rows (t*128+p), cols (h*512+c)
    shards = []
    for i in range(NCORES):
        arr = res.results[i]["out_sh"]  # [NH, 2, 128, NB]
        shards.append(arr.transpose(1, 2, 0, 3).reshape(2 * 128, NH * NB))
    xhatT = np.concatenate(shards, axis=0)  # [LD, B]
    return np.ascontiguousarray(xhatT.T).reshape(B, L, D).astype(np.float32)


# revision 7
# speedup vs baseline: 1.3579x; 1.0540x over previous
"""CrossCoder kernel for 8 Trainium2 NeuronCores (Bass/Tile, SPMD).

Math (reference):
    f     = relu(einsum('bld,ldf->bf', x, W_enc) + b_enc)     # [B, F]
    x_hat = einsum('bf,lfd->bld', f, W_dec) + b_dec           # [B, L, D]

Sharding: dict dim F=32768 split 8 ways (FL=4096 per core, tensor parallel
over latents). Each core computes its local f shard (encode) and the
partial decode sum over its latents; ReduceScatters combine the partials,
leaving each core with a distinct slice of the (LD=2048, B) transposed
output, which the host reassembles and transposes back.

Device layout is feature-major (contraction dim on SBUF partitions); batch
runs in two halves of 512 inside ONE TileContext. Collectives are emitted
in-context: RS0 after half 0 overlaps all of half 1; half 1's partial is
split in two (ld rows 0-1023 / 1024-2047) so RS1a overlaps the tail of the
decode and only RS1b (2MB) is exposed. Weights/x are host-repacked into
contiguous [128, 512] tiles so every DMA is one 256KB contiguous block.
b_dec/8 is folded in pre-collective. All matmuls are float32r (full PE
rate, ~2e-4 rel err).
"""

import numpy as np

B = 1024
L = 2
D = 1024
F = 32768
NCORES = 8
FL = F // NCORES      # 4096 latents per core
LD = L * D            # 2048
KT = LD // 128        # 16 encode k-tiles
FT = FL // 128        # 32 f-tiles per core
NB = 512              # matmul moving free dim
NH = 2                # batch halves

_CACHE = {}


def _build_nc():
    import concourse.bass as bass  # noqa: F401
    import concourse.tile as tile
    from concourse import bacc, mybir

    f32 = mybir.dt.float32
    f32r = mybir.dt.float32r

    nc = bacc.Bacc()

    xT = nc.declare_dram_parameter("xT", [NH, KT, 128, NB], f32r, isOutput=False)
    w_enc = nc.declare_dram_parameter("w_enc", [KT, FT // 4, 128, NB], f32r, isOutput=False)
    w_dec = nc.declare_dram_parameter("w_dec", [L, 2, FT, 128, NB], f32r, isOutput=False)
    b_enc = nc.declare_dram_parameter("b_enc", [FL], f32, isOutput=False)
    b_dec8 = nc.declare_dram_parameter("b_dec8", [LD], f32, isOutput=False)
    # out_sh: [0:2] = h0 ld-tiles {2i,2i+1}; [2] = h1 ld-tile i; [3] = h1 ld-tile 8+i
    out_sh = nc.declare_dram_parameter("out_sh", [4, 128, NB], f32, isOutput=True)

    # partial buffers: h0 is one 16-block tensor; h1 split by l for finer RS overlap
    partial0 = nc.dram_tensor("partial0", [KT, 128, NB], f32)
    partial1a = nc.dram_tensor("partial1a", [KT // 2, 128, NB], f32)
    partial1b = nc.dram_tensor("partial1b", [KT // 2, 128, NB], f32)
    rs0 = nc.dram_tensor("rs0", [2, 128, NB], f32)
    rs1a = nc.dram_tensor("rs1a", [1, 128, NB], f32)
    rs1b = nc.dram_tensor("rs1b", [1, 128, NB], f32)

    xT_a = xT.ap()
    w_enc_a = w_enc.ap()
    w_dec_a = w_dec.ap()
    rgroups = [list(range(NCORES))]

    with tile.TileContext(nc) as tc:
        with (
            tc.tile_pool(name="xp", bufs=1) as xp,
            tc.tile_pool(name="fp", bufs=1) as fp,
            tc.tile_pool(name="we", bufs=14) as we,
            tc.tile_pool(name="wd", bufs=14) as wd,
            tc.tile_pool(name="stg", bufs=8) as stg,
            tc.tile_pool(name="bias", bufs=1) as bias,
            tc.tile_pool(name="ps", bufs=8, space="PSUM") as ps,
        ):
            benc_t = bias.tile([128, FT], f32, name="benc")
            nc.sync.dma_start(
                out=benc_t, in_=b_enc.ap().rearrange("(t p) -> p t", p=128)
            )
            bdec_t = bias.tile([128, KT], f32, name="bdec")
            nc.sync.dma_start(
                out=bdec_t, in_=b_dec8.ap().rearrange("(t p) -> p t", p=128)
            )

            for h in range(NH):
                x_tiles = []
                for k in range(KT):
                    xt = xp.tile([128, NB], f32r, tag=f"x{k}", name=f"x{k}")
                    nc.sync.dma_start(out=xt, in_=xT_a[h, k])
                    x_tiles.append(xt)

                # ---- encode
                f_tiles = []
                for fg in range(FT // 4):
                    pss = [
                        ps.tile([128, NB], f32, tag="ps", name=f"pse{_j}")
                        for _j in range(4)
                    ]
                    for k in range(KT):
                        wt = we.tile([128, NB], f32r, tag="we", name="wet")
                        nc.sync.dma_start(out=wt, in_=w_enc_a[k, fg])
                        for j in range(4):
                            nc.tensor.matmul(
                                pss[j],
                                wt[:, j * 128 : (j + 1) * 128],
                                x_tiles[k],
                                start=(k == 0),
                                stop=(k == KT - 1),
                            )
                    for j in range(4):
                        ft_idx = fg * 4 + j
                        ftile = fp.tile(
                            [128, NB], f32r, tag=f"f{ft_idx}", name=f"f{ft_idx}"
                        )
                        nc.scalar.activation(
                            ftile,
                            pss[j],
                            mybir.ActivationFunctionType.Relu,
                            bias=benc_t[:, ft_idx : ft_idx + 1],
                        )
                        f_tiles.append(ftile)

                # ---- decode
                for l in range(L):
                    if h == 0:
                        part_a = partial0.ap()
                        base = l * 8
                    else:
                        part_a = (partial1a if l == 0 else partial1b).ap()
                        base = 0
                    for dg in range(2):
                        pss = [
                            ps.tile([128, NB], f32, tag="ps", name=f"psd{_j}")
                            for _j in range(4)
                        ]
                        for fk in range(FT):
                            wt = wd.tile([128, NB], f32r, tag="wd", name="wdt")
                            nc.sync.dma_start(out=wt, in_=w_dec_a[l, dg, fk])
                            for j in range(4):
                                nc.tensor.matmul(
                                    pss[j],
                                    wt[:, j * 128 : (j + 1) * 128],
                                    f_tiles[fk],
                                    start=(fk == 0),
                                    stop=(fk == FT - 1),
                                )
                        for j in range(4):
                            ld_t = l * 8 + dg * 4 + j
                            st = stg.tile([128, NB], f32, tag="st", name="st")
                            nc.vector.tensor_scalar_add(
                                st, pss[j], bdec_t[:, ld_t : ld_t + 1]
                            )
                            nc.sync.dma_start(
                                out=part_a[base + dg * 4 + j], in_=st
                            )
                    if h == 1:
                        # this l-block's partial is complete → ReduceScatter it
                        src, dst = (partial1a, rs1a) if l == 0 else (partial1b, rs1b)
                        nc.gpsimd.collective_compute(
                            "ReduceScatter",
                            mybir.AluOpType.add,
                            ins=[src[:]],
                            outs=[dst[:]],
                            replica_groups=rgroups,
                        )

                if h == 0:
                    nc.gpsimd.collective_compute(
                        "ReduceScatter",
                        mybir.AluOpType.add,
                        ins=[partial0[:]],
                        outs=[rs0[:]],
                        replica_groups=rgroups,
                    )

            out_a = out_sh.ap()
            nc.gpsimd.dma_start(out=out_a[0:2], in_=rs0[:])
            nc.gpsimd.dma_start(out=out_a[2:3], in_=rs1a[:])
            nc.gpsimd.dma_start(out=out_a[3:4], in_=rs1b[:])

    nc.finalize()
    return nc


def _get_nc():
    if "nc" not in _CACHE:
        _CACHE["nc"] = _build_nc()
    return _CACHE["nc"]


def kernel(x, W_enc, b_enc, W_dec, b_dec):
    from concourse.bass_utils import run_bass_kernel_spmd

    x = np.asarray(x, dtype=np.float32)
    W_enc = np.asarray(W_enc, dtype=np.float32)
    b_enc = np.asarray(b_enc, dtype=np.float32)
    W_dec = np.asarray(W_dec, dtype=np.float32)
    b_dec = np.asarray(b_dec, dtype=np.float32)

    nc = _get_nc()

    # xT blocked: [h, k, p, c] with xT row k*128+p (= x.reshape(B,LD).T), col h*512+c
    xT = np.ascontiguousarray(
        x.reshape(B, LD).T.reshape(KT, 128, NH, NB).transpose(2, 0, 1, 3)
    )
    w_enc_flat = W_enc.reshape(LD, F)
    bdec8 = (b_dec.reshape(LD) / NCORES).astype(np.float32)

    in_maps = []
    for i in range(NCORES):
        fsl = slice(i * FL, (i + 1) * FL)
        we_blk = np.ascontiguousarray(
            w_enc_flat[:, fsl].reshape(KT, 128, FT // 4, NB).transpose(0, 2, 1, 3)
        )
        wd_blk = np.ascontiguousarray(
            W_dec[:, fsl, :].reshape(L, FT, 128, 2, NB).transpose(0, 3, 1, 2, 4)
        )
        in_maps.append(
            {
                "xT": xT,
                "w_enc": we_blk,
                "w_dec": wd_blk,
                "b_enc": np.ascontiguousarray(b_enc[fsl]),
                "b_dec8": bdec8,
            }
        )

    res = run_bass_kernel_spmd(nc, in_maps, list(range(NCORES)))
    _CACHE["last_res"] = res

    xhatT = np.empty((LD, B), dtype=np.float32)
    for i in range(NCORES):
        arr = res.results[i]["out_sh"]  # [4, 128, NB]
        xhatT[2 * i * 128 : (2 * i + 2) * 128, 0:NB] = arr[0:2].reshape(256, NB)
        xhatT[i * 128 : (i + 1) * 128, NB : 2 * NB] = arr[2]
        xhatT[(8 + i) * 128 : (9 + i) * 128, NB : 2 * NB] = arr[3]
    return np.ascontiguousarray(xhatT.T).reshape(B, L, D).astype(np.float32)


# revision 8
# speedup vs baseline: 1.3583x; 1.0003x over previous
"""CrossCoder kernel for 8 Trainium2 NeuronCores (Bass/Tile, SPMD).

Math (reference):
    f     = relu(einsum('bld,ldf->bf', x, W_enc) + b_enc)     # [B, F]
    x_hat = einsum('bf,lfd->bld', f, W_dec) + b_dec           # [B, L, D]

Sharding: dict dim F=32768 split 8 ways (FL=4096 per core, tensor parallel
over latents). Each core computes its local f shard (encode) and the
partial decode sum over its latents; ReduceScatters combine the partials,
leaving each core with a distinct slice of the (LD=2048, B) transposed
output, which the host reassembles and transposes back.

Device layout is feature-major (contraction dim on SBUF partitions); batch
runs in two halves of 512 inside ONE TileContext. Collectives are emitted
in-context: RS0 after half 0 overlaps all of half 1; half 1's partial is
split in two (ld rows 0-1023 / 1024-2047) so RS1a overlaps the tail of the
decode and only RS1b (2MB) is exposed. Weights/x are host-repacked into
contiguous [128, 512] tiles so every DMA is one 256KB contiguous block.
b_dec/8 is folded in pre-collective. All matmuls are float32r (full PE
rate, ~2e-4 rel err).
"""

import numpy as np

B = 1024
L = 2
D = 1024
F = 32768
NCORES = 8
FL = F // NCORES      # 4096 latents per core
LD = L * D            # 2048
KT = LD // 128        # 16 encode k-tiles
FT = FL // 128        # 32 f-tiles per core
NB = 512              # matmul moving free dim
NH = 2                # batch halves

_CACHE = {}


def _build_nc():
    import concourse.bass as bass  # noqa: F401
    import concourse.tile as tile
    from concourse import bacc, mybir

    f32 = mybir.dt.float32
    f32r = mybir.dt.float32r

    nc = bacc.Bacc()

    xT = nc.declare_dram_parameter("xT", [NH, KT, 128, NB], f32r, isOutput=False)
    w_enc = nc.declare_dram_parameter("w_enc", [KT, FT // 4, 128, NB], f32r, isOutput=False)
    w_dec = nc.declare_dram_parameter("w_dec", [L, 2, FT, 128, NB], f32r, isOutput=False)
    b_enc = nc.declare_dram_parameter("b_enc", [FL], f32, isOutput=False)
    b_dec8 = nc.declare_dram_parameter("b_dec8", [LD], f32, isOutput=False)
    # out_sh: [0:2] = h0 ld-tiles {2i,2i+1}; [2] = h1 ld-tile i; [3] = h1 ld-tile 8+i
    out_sh = nc.declare_dram_parameter("out_sh", [4, 128, NB], f32, isOutput=True)

    # partial buffers: h0 is one 16-block tensor; h1 split by l for finer RS overlap
    partial0 = nc.dram_tensor("partial0", [KT, 128, NB], f32)
    partial1a = nc.dram_tensor("partial1a", [KT // 2, 128, NB], f32)
    partial1b = nc.dram_tensor("partial1b", [KT // 2, 128, NB], f32)
    rs0 = nc.dram_tensor("rs0", [2, 128, NB], f32)
    rs1a = nc.dram_tensor("rs1a", [1, 128, NB], f32)
    rs1b = nc.dram_tensor("rs1b", [1, 128, NB], f32)

    xT_a = xT.ap()
    w_enc_a = w_enc.ap()
    w_dec_a = w_dec.ap()
    rgroups = [list(range(NCORES))]

    with tile.TileContext(nc) as tc:
        with (
            tc.tile_pool(name="xp", bufs=1) as xp,
            tc.tile_pool(name="fp", bufs=1) as fp,
            tc.tile_pool(name="we", bufs=14) as we,
            tc.tile_pool(name="wd", bufs=14) as wd,
            tc.tile_pool(name="stg", bufs=6) as stg,
            tc.tile_pool(name="bias", bufs=1) as bias,
            tc.tile_pool(name="ps", bufs=8, space="PSUM") as ps,
        ):
            benc_t = bias.tile([128, FT], f32, name="benc")
            nc.sync.dma_start(
                out=benc_t, in_=b_enc.ap().rearrange("(t p) -> p t", p=128)
            )
            bdec_t = bias.tile([128, KT], f32, name="bdec")
            nc.sync.dma_start(
                out=bdec_t, in_=b_dec8.ap().rearrange("(t p) -> p t", p=128)
            )

            for h in range(NH):
                x_tiles = []
                for k in range(KT):
                    xt = xp.tile([128, NB], f32r, tag=f"x{k}", name=f"x{k}")
                    nc.sync.dma_start(out=xt, in_=xT_a[h, k])
                    x_tiles.append(xt)

                # ---- encode
                f_tiles = []
                for fg in range(FT // 4):
                    pss = [
                        ps.tile([128, NB], f32, tag="ps", name=f"pse{_j}")
                        for _j in range(4)
                    ]
                    for k in range(KT):
                        wt = we.tile([128, NB], f32r, tag="we", name="wet")
                        nc.sync.dma_start(out=wt, in_=w_enc_a[k, fg])
                        for j in range(4):
                            nc.tensor.matmul(
                                pss[j],
                                wt[:, j * 128 : (j + 1) * 128],
                                x_tiles[k],
                                start=(k == 0),
                                stop=(k == KT - 1),
                            )
                    for j in range(4):
                        ft_idx = fg * 4 + j
                        ftile = fp.tile(
                            [128, NB], f32r, tag=f"f{ft_idx}", name=f"f{ft_idx}",
                            bufs=2 if ft_idx < 8 else 1,
                        )
                        nc.scalar.activation(
                            ftile,
                            pss[j],
                            mybir.ActivationFunctionType.Relu,
                            bias=benc_t[:, ft_idx : ft_idx + 1],
                        )
                        f_tiles.append(ftile)

                # ---- decode
                for l in range(L):
                    if h == 0:
                        part_a = partial0.ap()
                        base = l * 8
                    else:
                        part_a = (partial1a if l == 0 else partial1b).ap()
                        base = 0
                    for dg in range(2):
                        pss = [
                            ps.tile([128, NB], f32, tag="ps", name=f"psd{_j}")
                            for _j in range(4)
                        ]
                        for fk in range(FT):
                            wt = wd.tile([128, NB], f32r, tag="wd", name="wdt")
                            nc.sync.dma_start(out=wt, in_=w_dec_a[l, dg, fk])
                            for j in range(4):
                                nc.tensor.matmul(
                                    pss[j],
                                    wt[:, j * 128 : (j + 1) * 128],
                                    f_tiles[fk],
                                    start=(fk == 0),
                                    stop=(fk == FT - 1),
                                )
                        for j in range(4):
                            ld_t = l * 8 + dg * 4 + j
                            st = stg.tile([128, NB], f32, tag="st", name="st")
                            nc.vector.tensor_scalar_add(
                                st, pss[j], bdec_t[:, ld_t : ld_t + 1]
                            )
                            nc.sync.dma_start(
                                out=part_a[base + dg * 4 + j], in_=st
                            )
                    if h == 1:
                        # this l-block's partial is complete → ReduceScatter it
                        src, dst = (partial1a, rs1a) if l == 0 else (partial1b, rs1b)
                        nc.gpsimd.collective_compute(
                            "ReduceScatter",
                            mybir.AluOpType.add,
                            ins=[src[:]],
                            outs=[dst[:]],
                            replica_groups=rgroups,
                        )

                if h == 0:
                    nc.gpsimd.collective_compute(
                        "ReduceScatter",
                        mybir.AluOpType.add,
                        ins=[partial0[:]],
                        outs=[rs0[:]],
                        replica_groups=rgroups,
                    )

            out_a = out_sh.ap()
            nc.gpsimd.dma_start(out=out_a[0:2], in_=rs0[:])
            nc.gpsimd.dma_start(out=out_a[2:3], in_=rs1a[:])
            nc.gpsimd.dma_start(out=out_a[3:4], in_=rs1b[:])

    nc.finalize()
    return nc


def _get_nc():
    if "nc" not in _CACHE:
        _CACHE["nc"] = _build_nc()
    return _CACHE["nc"]


def kernel(x, W_enc, b_enc, W_dec, b_dec):
    from concourse.bass_utils import run_bass_kernel_spmd

    x = np.asarray(x, dtype=np.float32)
    W_enc = np.asarray(W_enc, dtype=np.float32)
    b_enc = np.asarray(b_enc, dtype=np.float32)
    W_dec = np.asarray(W_dec, dtype=np.float32)
    b_dec = np.asarray(b_dec, dtype=np.float32)

    nc = _get_nc()

    # xT blocked: [h, k, p, c] with xT row k*128+p (= x.reshape(B,LD).T), col h*512+c
    xT = np.ascontiguousarray(
        x.reshape(B, LD).T.reshape(KT, 128, NH, NB).transpose(2, 0, 1, 3)
    )
    w_enc_flat = W_enc.reshape(LD, F)
    bdec8 = (b_dec.reshape(LD) / NCORES).astype(np.float32)

    in_maps = []
    for i in range(NCORES):
        fsl = slice(i * FL, (i + 1) * FL)
        we_blk = np.ascontiguousarray(
            w_enc_flat[:, fsl].reshape(KT, 128, FT // 4, NB).transpose(0, 2, 1, 3)
        )
        wd_blk = np.ascontiguousarray(
            W_dec[:, fsl, :].reshape(L, FT, 128, 2, NB).transpose(0, 3, 1, 2, 4)
        )
        in_maps.append(
            {
                "xT": xT,
                "w_enc": we_blk,
                "w_dec": wd_blk,
                "b_enc": np.ascontiguousarray(b_enc[fsl]),
                "b_dec8": bdec8,
            }
        )

    res = run_bass_kernel_spmd(nc, in_maps, list(range(NCORES)))
    _CACHE["last_res"] = res

    xhatT = np.empty((LD, B), dtype=np.float32)
    for i in range(NCORES):
        arr = res.results[i]["out_sh"]  # [4, 128, NB]
        xhatT[2 * i * 128 : (2 * i + 2) * 128, 0:NB] = arr[0:2].reshape(256, NB)
        xhatT[i * 128 : (i + 1) * 128, NB : 2 * NB] = arr[2]
        xhatT[(8 + i) * 128 : (9 + i) * 128, NB : 2 * NB] = arr[3]
    return np.ascontiguousarray(xhatT.T).reshape(B, L, D).astype(np.float32)
